# revision 14
# baseline (speedup 1.0000x reference)
"""Trainium2 kernel: out[n, k] = prod_c vector_list[n, c] ** l_list[k, c].

Data-parallel over 8 NeuronCores: vector_list is sharded along the row
dimension; the exponent table l_list is baked into the instruction stream
host-side (exponents are small non-negative ints), so each output column
is one elementwise op (mul / square / copy / memset) reading previously
computed columns in SBUF.

Layout per core: rows are tiled [128 partitions, F rows, .] so both the
input load and the output store are fully contiguous DRAM transfers; the
per-column compute ops use stride-L views of the output tile.
"""

import math
import sys

import numpy as np

sys.path.insert(0, "/opt/trn_rl_repo")

P = 128          # SBUF partitions
F_MAX = 280      # max rows-per-partition per chunk
N_CORES = 8
GP_MULS = 0      # leaf muls offloaded from VectorE to GpSimd (~23 cyc/elem, never worth it)


def _plan(exps):
    """Codegen plan for computing all monomial columns.

    exps: list of (lx, ly, lz) int tuples.
    Returns (steps, n_scratch) where steps reference value symbols:
      ('basis', c) — input component c as a strided view
      ('out', k)   — output column k
      ('scr', i)   — scratch column i
    Step kinds:
      ('one', dst) | ('copy', dst, src) | ('square', dst, src)
      | ('mul', dst, a, b)
    """
    basis = {(1, 0, 0): ("basis", 0), (0, 1, 0): ("basis", 1), (0, 0, 1): ("basis", 2)}
    avail = {}
    steps = []
    nscr = 0

    def sub(e, u):
        return (e[0] - u[0], e[1] - u[1], e[2] - u[2])

    def lookup(t):
        if t in avail:
            return avail[t]
        if t in basis:
            return basis[t]
        return None

    def get(e):
        nonlocal nscr
        v = lookup(e)
        if v is not None:
            return v
        dst = ("scr", nscr)
        nscr += 1
        emit(dst, e)
        return dst

    def emit(dst, e):
        # Prefer an ACT square (offloads the vector engine) when e is even
        # and its half is already materialized.
        if all(c % 2 == 0 for c in e):
            h = tuple(c // 2 for c in e)
            if lookup(h) is not None:
                steps.append(("square", dst, lookup(h)))
                avail[e] = dst
                return
        # Split into two already-available factors.
        for a in list(avail.keys()) + list(basis.keys()):
            if sum(a) == 0 or sub(e, a) == e:
                continue
            b = sub(e, a)
            if min(b) < 0 or sum(b) == 0:
                continue
            va, vb = lookup(a), lookup(b)
            if va is not None and vb is not None:
                steps.append(("mul", dst, va, vb))
                avail[e] = dst
                return
        # Even split via square of a recursively built half.
        if all(c % 2 == 0 for c in e):
            h = tuple(c // 2 for c in e)
            steps.append(("square", dst, get(h)))
            avail[e] = dst
            return
        # Peel a unit factor off the largest axis.
        ax = max(range(3), key=lambda i: e[i])
        u = tuple(1 if i == ax else 0 for i in range(3))
        rest = get(sub(e, u))
        steps.append(("mul", dst, rest, basis[u]))
        avail[e] = dst

    order = sorted(range(len(exps)), key=lambda k: (sum(exps[k]), exps[k]))
    for k in order:
        e = tuple(exps[k])
        dst = ("out", k)
        if sum(e) == 0:
            steps.append(("one", dst))
            continue
        if e in avail:
            steps.append(("copy", dst, avail[e]))
            continue
        if e in basis:
            # Materialize the column, but keep reading the basis view for
            # later products (avoids a cross-op dependency on the copy).
            steps.append(("copy", dst, basis[e]))
            avail[e] = basis[e]
            continue
        emit(dst, e)
    return steps, nscr


def _build(R, exps):
    """Build the per-core Bacc graph for R rows (R divisible by P)."""
    import concourse.bass as bass  # noqa: F401  (engine types)
    import concourse.tile as tile
    from concourse import bacc, mybir

    L = len(exps)
    steps, nscr = _plan(exps)
    f32 = mybir.dt.float32

    # Engine balance (measured per-op costs at F~280: DVE mul 570ns,
    # DVE copy 274ns, ACT square/copy 705ns, GpSimd 1-input ~line rate):
    # squares -> ACT, memset/copies -> GpSimd, muls -> DVE except a few
    # leaf muls (results never read again) offloaded to GpSimd.
    used = set()
    for st in steps:
        for sym in st[2:]:
            used.add(sym)
    gp_muls = set()
    for i in reversed(range(len(steps))):
        st = steps[i]
        if len(gp_muls) >= GP_MULS:
            break
        if st[0] == "mul" and st[1] not in used:
            gp_muls.add(i)

    nc = bacc.Bacc()
    vec = nc.declare_dram_parameter("vector_list", [R, 3], f32, isOutput=False)
    out = nc.declare_dram_parameter("out", [R, L], f32, isOutput=True)

    rows_p = R // P
    n_chunks = max(1, math.ceil(rows_p / F_MAX))
    base, rem = divmod(rows_p, n_chunks)
    sizes = [base + 1] * rem + [base] * (n_chunks - rem)
    for _ in range(2):
        # Taper twice: the first and last chunks shrink to ~quarter size so
        # the store pipeline ramps up quickly and drains quickly.
        if len(sizes) >= 2:
            f, l = sizes[0], sizes[-1]
            sizes = [f // 2, f - f // 2] + sizes[1:-1] + [l // 2, l - l // 2]
            sizes = [s for s in sizes if s > 0]

    with tile.TileContext(nc) as tc:
        with (
            tc.tile_pool(name="inp", bufs=4) as inp,
            tc.tile_pool(name="outp", bufs=5) as outp,
            tc.tile_pool(name="scrp", bufs=2) as scrp,
        ):
            r0 = 0
            for F in sizes:
                rows = P * F
                tin = inp.tile([P, F, 3], f32)
                nc.scalar.dma_start(
                    tin[:],
                    vec[r0 : r0 + rows, :].rearrange("(p f) c -> p f c", p=P),
                )
                tout = outp.tile([P, F, L], f32)
                tscr = scrp.tile([P, F, nscr], f32) if nscr else None

                def ap(sym):
                    kind, i = sym
                    if kind == "basis":
                        return tin[:, :, i]
                    if kind == "out":
                        return tout[:, :, i]
                    return tscr[:, :, i]

                for i, st in enumerate(steps):
                    if st[0] == "one":
                        nc.gpsimd.memset(ap(st[1]), 1.0)
                    elif st[0] == "copy":
                        nc.scalar.copy(ap(st[1]), ap(st[2]))
                    elif st[0] == "square":
                        nc.scalar.square(ap(st[1]), ap(st[2]))
                    elif i in gp_muls:
                        nc.gpsimd.tensor_mul(ap(st[1]), ap(st[2]), ap(st[3]))
                    else:
                        nc.vector.tensor_mul(ap(st[1]), ap(st[2]), ap(st[3]))

                nc.sync.dma_start(
                    out[r0 : r0 + rows, :].rearrange("(p f) k -> p f k", p=P),
                    tout[:],
                )
                r0 += rows
    nc.finalize()
    return nc


_CACHE = {}
_LAST_RESULT = None  # BassKernelResults of the most recent run (for profiling)


def kernel(vector_list: np.ndarray, l_list: np.ndarray) -> np.ndarray:
    from concourse.bass_utils import run_bass_kernel_spmd

    vector_list = np.ascontiguousarray(vector_list, dtype=np.float32)
    l_list = np.asarray(l_list)
    N = vector_list.shape[0]
    L = l_list.shape[0]
    exps = tuple(tuple(int(v) for v in row) for row in l_list)

    rows_unit = N_CORES * P
    n_dev = (N // rows_unit) * rows_unit
    R = n_dev // N_CORES

    outv = np.empty((N, L), dtype=np.float32)
    if R > 0:
        key = (R, exps)
        if key not in _CACHE:
            _CACHE[key] = _build(R, exps)
        nc = _CACHE[key]
        in_maps = [
            {"vector_list": vector_list[i * R : (i + 1) * R]} for i in range(N_CORES)
        ]
        res = run_bass_kernel_spmd(nc, in_maps, core_ids=list(range(N_CORES)))
        global _LAST_RESULT
        _LAST_RESULT = res
        for i in range(N_CORES):
            outv[i * R : (i + 1) * R] = res.results[i]["out"]
    if n_dev < N:
        tail = vector_list[n_dev:]
        le = np.asarray(l_list, dtype=np.float32)
        outv[n_dev:] = np.prod(
            tail[:, None, :] ** le[None, :, :], axis=-1, dtype=np.float32
        )
    return outv


# revision 15
# speedup vs baseline: 1.0214x; 1.0214x over previous
"""Trainium2 kernel: out[n, k] = prod_c vector_list[n, c] ** l_list[k, c].

Data-parallel over 8 NeuronCores: vector_list is sharded along the row
dimension; the exponent table l_list is baked into the instruction stream
host-side (exponents are small non-negative ints), so each output column
is one elementwise op (mul / square / copy / memset) reading previously
computed columns in SBUF.

Layout per core: rows are tiled [128 partitions, F rows, .] so both the
input load and the output store are fully contiguous DRAM transfers; the
per-column compute ops use stride-L views of the output tile.
"""

import math
import sys

import numpy as np

sys.path.insert(0, "/opt/trn_rl_repo")

P = 128          # SBUF partitions
F_MAX = 280      # max rows-per-partition per chunk
N_CORES = 8
GP_MULS = 0      # leaf muls offloaded from VectorE to GpSimd (~23 cyc/elem, never worth it)


def _plan(exps):
    """Codegen plan for computing all monomial columns.

    exps: list of (lx, ly, lz) int tuples.
    Returns (steps, n_scratch) where steps reference value symbols:
      ('basis', c) — input component c as a strided view
      ('out', k)   — output column k
      ('scr', i)   — scratch column i
    Step kinds:
      ('one', dst) | ('copy', dst, src) | ('square', dst, src)
      | ('mul', dst, a, b)
    """
    basis = {(1, 0, 0): ("basis", 0), (0, 1, 0): ("basis", 1), (0, 0, 1): ("basis", 2)}
    avail = {}
    steps = []
    nscr = 0

    def sub(e, u):
        return (e[0] - u[0], e[1] - u[1], e[2] - u[2])

    def lookup(t):
        if t in avail:
            return avail[t]
        if t in basis:
            return basis[t]
        return None

    def get(e):
        nonlocal nscr
        v = lookup(e)
        if v is not None:
            return v
        dst = ("scr", nscr)
        nscr += 1
        emit(dst, e)
        return dst

    def emit(dst, e):
        # Prefer an ACT square (offloads the vector engine) when e is even
        # and its half is already materialized.
        if all(c % 2 == 0 for c in e):
            h = tuple(c // 2 for c in e)
            if lookup(h) is not None:
                steps.append(("square", dst, lookup(h)))
                avail[e] = dst
                return
        # Split into two already-available factors.
        for a in list(avail.keys()) + list(basis.keys()):
            if sum(a) == 0 or sub(e, a) == e:
                continue
            b = sub(e, a)
            if min(b) < 0 or sum(b) == 0:
                continue
            va, vb = lookup(a), lookup(b)
            if va is not None and vb is not None:
                steps.append(("mul", dst, va, vb))
                avail[e] = dst
                return
        # Even split via square of a recursively built half.
        if all(c % 2 == 0 for c in e):
            h = tuple(c // 2 for c in e)
            steps.append(("square", dst, get(h)))
            avail[e] = dst
            return
        # Peel a unit factor off the largest axis.
        ax = max(range(3), key=lambda i: e[i])
        u = tuple(1 if i == ax else 0 for i in range(3))
        rest = get(sub(e, u))
        steps.append(("mul", dst, rest, basis[u]))
        avail[e] = dst

    order = sorted(range(len(exps)), key=lambda k: (sum(exps[k]), exps[k]))
    for k in order:
        e = tuple(exps[k])
        dst = ("out", k)
        if sum(e) == 0:
            steps.append(("one", dst))
            continue
        if e in avail:
            steps.append(("copy", dst, avail[e]))
            continue
        if e in basis:
            # Materialize the column, but keep reading the basis view for
            # later products (avoids a cross-op dependency on the copy).
            steps.append(("copy", dst, basis[e]))
            avail[e] = basis[e]
            continue
        emit(dst, e)
    return steps, nscr


def _build(R, exps):
    """Build the per-core Bacc graph for R rows (R divisible by P)."""
    import concourse.bass as bass  # noqa: F401  (engine types)
    import concourse.tile as tile
    from concourse import bacc, mybir

    L = len(exps)
    steps, nscr = _plan(exps)
    f32 = mybir.dt.float32

    # Engine balance (measured per-op costs at F~280: DVE mul 570ns,
    # DVE copy 274ns, ACT square/copy 705ns, GpSimd 1-input ~line rate):
    # squares -> ACT, memset/copies -> GpSimd, muls -> DVE except a few
    # leaf muls (results never read again) offloaded to GpSimd.
    used = set()
    for st in steps:
        for sym in st[2:]:
            used.add(sym)
    gp_muls = set()
    for i in reversed(range(len(steps))):
        st = steps[i]
        if len(gp_muls) >= GP_MULS:
            break
        if st[0] == "mul" and st[1] not in used:
            gp_muls.add(i)

    nc = bacc.Bacc()
    vec = nc.declare_dram_parameter("vector_list", [R, 3], f32, isOutput=False)
    out = nc.declare_dram_parameter("out", [R, L], f32, isOutput=True)

    rows_p = R // P
    n_chunks = max(1, math.ceil(rows_p / F_MAX))
    base, rem = divmod(rows_p, n_chunks)
    sizes = [base + 1] * rem + [base] * (n_chunks - rem)
    if len(sizes) >= 2:
        # Taper: split the first and last chunks so the store pipeline
        # starts earlier and drains faster.
        f, l = sizes[0], sizes[-1]
        sizes = [f // 2, f - f // 2] + sizes[1:-1] + [l // 2, l - l // 2]
        sizes = [s for s in sizes if s > 0]

    with tile.TileContext(nc) as tc:
        with (
            tc.tile_pool(name="inp", bufs=4) as inp,
            tc.tile_pool(name="outp", bufs=4) as outp,
            tc.tile_pool(name="scrp", bufs=2) as scrp,
        ):
            r0 = 0
            for F in sizes:
                rows = P * F
                tin = inp.tile([P, F, 3], f32)
                nc.scalar.dma_start(
                    tin[:],
                    vec[r0 : r0 + rows, :].rearrange("(p f) c -> p f c", p=P),
                )
                tout = outp.tile([P, F, L], f32)
                tscr = scrp.tile([P, F, nscr], f32) if nscr else None

                def ap(sym):
                    kind, i = sym
                    if kind == "basis":
                        return tin[:, :, i]
                    if kind == "out":
                        return tout[:, :, i]
                    return tscr[:, :, i]

                for i, st in enumerate(steps):
                    if st[0] == "one":
                        nc.gpsimd.memset(ap(st[1]), 1.0)
                    elif st[0] == "copy":
                        nc.scalar.copy(ap(st[1]), ap(st[2]))
                    elif st[0] == "square":
                        nc.scalar.square(ap(st[1]), ap(st[2]))
                    elif i in gp_muls:
                        nc.gpsimd.tensor_mul(ap(st[1]), ap(st[2]), ap(st[3]))
                    else:
                        nc.vector.tensor_mul(ap(st[1]), ap(st[2]), ap(st[3]))

                nc.sync.dma_start(
                    out[r0 : r0 + rows, :].rearrange("(p f) k -> p f k", p=P),
                    tout[:],
                )
                r0 += rows
    nc.finalize()
    return nc


_CACHE = {}
_LAST_RESULT = None  # BassKernelResults of the most recent run (for profiling)


def kernel(vector_list: np.ndarray, l_list: np.ndarray) -> np.ndarray:
    from concourse.bass_utils import run_bass_kernel_spmd

    vector_list = np.ascontiguousarray(vector_list, dtype=np.float32)
    l_list = np.asarray(l_list)
    N = vector_list.shape[0]
    L = l_list.shape[0]
    exps = tuple(tuple(int(v) for v in row) for row in l_list)

    rows_unit = N_CORES * P
    n_dev = (N // rows_unit) * rows_unit
    R = n_dev // N_CORES

    outv = np.empty((N, L), dtype=np.float32)
    if R > 0:
        key = (R, exps)
        if key not in _CACHE:
            _CACHE[key] = _build(R, exps)
        nc = _CACHE[key]
        in_maps = [
            {"vector_list": vector_list[i * R : (i + 1) * R]} for i in range(N_CORES)
        ]
        res = run_bass_kernel_spmd(nc, in_maps, core_ids=list(range(N_CORES)))
        global _LAST_RESULT
        _LAST_RESULT = res
        for i in range(N_CORES):
            outv[i * R : (i + 1) * R] = res.results[i]["out"]
    if n_dev < N:
        tail = vector_list[n_dev:]
        le = np.asarray(l_list, dtype=np.float32)
        outv[n_dev:] = np.prod(
            tail[:, None, :] ** le[None, :, :], axis=-1, dtype=np.float32
        )
    return outv


# revision 18
# speedup vs baseline: 1.1115x; 1.0882x over previous
"""Trainium2 kernel: out[n, k] = prod_c vector_list[n, c] ** l_list[k, c].

Data-parallel over 8 NeuronCores: vector_list is sharded along the row
dimension; the exponent table l_list is baked into the instruction stream
host-side (exponents are small non-negative ints), so each output column
is one elementwise op (mul / square / copy / memset) reading previously
computed values in SBUF.

Layout per core: rows are tiled [128 partitions, F rows, .] so both the
input load and the output store are fully contiguous DRAM transfers.
Values that are re-used as operands ("non-leaf") are computed into a
contiguous k-major scratch tile (strided SBUF streams halve VectorE
throughput, so products of contiguous streams are ~1.6x faster) and then
materialized into the strided output column by a cheap copy; leaf values
are computed straight into their output column.
"""

import math
import sys

import numpy as np

sys.path.insert(0, "/opt/trn_rl_repo")

P = 128          # SBUF partitions
F_MAX = 280      # max rows-per-partition per chunk
N_CORES = 8

# Measured per-op costs (ns) at F=279 on TRN2, used for static load balance.
_DVE_MUL_CC = 354     # mul, contiguous out, contiguous ins
_DVE_MUL_SC = 514     # mul, strided out, contiguous ins
_DVE_MUL_SS = 570     # mul, strided out/ins
_DVE_COPY = 265       # copy, one strided stream
_ACT_SQ = 560         # ACT square
_ACT_COPY = 600       # ACT copy with a strided stream


def _plan(exps):
    """Codegen plan.

    Value symbols:
      ('basis', c) — input component c (stride-3 view of the input tile)
      ('out', k)   — output column k (stride-L view of the output tile)
      ('scr', i)   — scratch column i (contiguous view of the scratch tile)

    Returns (steps, col_of):
      steps: ops in dependency order, dsts all scratch symbols:
        ('copy', dst, src) | ('square', dst, src) | ('mul', dst, a, b)
      col_of: dict output-column -> sym holding its final value, where
        ('one',) marks the constant-1 column.
    """
    basis = {(1, 0, 0): ("basis", 0), (0, 1, 0): ("basis", 1), (0, 0, 1): ("basis", 2)}
    avail = {}    # exponent-tuple -> sym holding its value
    steps = []
    nscr = 0

    def sub(e, u):
        return (e[0] - u[0], e[1] - u[1], e[2] - u[2])

    def lookup(t):
        if t in avail:
            return avail[t]
        if t in basis:
            return basis[t]
        return None

    def new_scr():
        nonlocal nscr
        s = ("scr", nscr)
        nscr += 1
        return s

    def get(e):
        v = lookup(e)
        if v is not None:
            return v
        dst = new_scr()
        emit(dst, e)
        return dst

    def emit(dst, e):
        # Prefer an ACT square when e is even and its half is available.
        if all(c % 2 == 0 for c in e):
            h = tuple(c // 2 for c in e)
            if lookup(h) is not None:
                steps.append(("square", dst, lookup(h)))
                avail[e] = dst
                return
        # Split into two already-available factors.
        for a in list(avail.keys()) + list(basis.keys()):
            if sum(a) == 0 or sub(e, a) == e:
                continue
            b = sub(e, a)
            if min(b) < 0 or sum(b) == 0:
                continue
            va, vb = lookup(a), lookup(b)
            if va is not None and vb is not None:
                steps.append(("mul", dst, va, vb))
                avail[e] = dst
                return
        # Even split via square of a recursively built half.
        if all(c % 2 == 0 for c in e):
            steps.append(("square", dst, get(tuple(c // 2 for c in e))))
            avail[e] = dst
            return
        # Peel a unit factor off the largest axis.
        ax = max(range(3), key=lambda i: e[i])
        u = tuple(1 if i == ax else 0 for i in range(3))
        rest = get(sub(e, u))
        steps.append(("mul", dst, rest, basis[u]))
        avail[e] = dst

    # Build every distinct monomial value, in degree order so sub-monomials
    # exist first.
    order = sorted(range(len(exps)), key=lambda k: (sum(exps[k]), exps[k]))
    col_of = {}
    for k in order:
        e = tuple(exps[k])
        if sum(e) == 0:
            col_of[k] = ("one",)
            continue
        col_of[k] = get(e)
    return steps, col_of


def _assign(steps, col_of):
    """Decide storage (scratch vs direct-to-column) and engines.

    A computed value read by a later step stays in scratch (contiguous =
    fast operands) and gets a materialization copy into its output column;
    a leaf value writes its column directly. Materialization copies are
    greedily balanced between VectorE and ScalarE by modeled cost.

    Returns (ops, n_scratch) where ops is a list of
    (engine, kind, dst_sym, src_syms...) with engine in {'v','a','g'}.
    """
    used = set()
    for st in steps:
        for sym in st[2:]:
            used.add(sym)

    cols_by_sym = {}
    for k, sym in col_of.items():
        if sym != ("one",):
            cols_by_sym.setdefault(sym, []).append(k)

    retarget = {}
    for st in steps:
        dst = st[1]
        if dst in used:
            continue
        cols = cols_by_sym.get(dst, [])
        if len(cols) == 1:
            retarget[dst] = ("out", cols[0])

    load = {"v": 0.0, "a": 0.0}
    ops = []
    for st in steps:
        kind, dst = st[0], st[1]
        tgt = retarget.get(dst, dst)
        strided_out = tgt[0] == "out"
        srcs = st[2:]
        if kind == "square":
            ops.append(("a", "square", tgt) + srcs)
            load["a"] += _ACT_SQ + (60 if strided_out else 0)
        elif kind == "mul":
            if any(s[0] == "out" for s in srcs):
                cost = _DVE_MUL_SS
            elif strided_out:
                cost = _DVE_MUL_SC
            else:
                cost = _DVE_MUL_CC
            ops.append(("v", "mul", tgt) + srcs)
            load["v"] += cost
        else:
            ops.append(("v", "copy", tgt) + srcs)
            load["v"] += _DVE_COPY

    # Materialization copies for output columns whose value lives elsewhere
    # (re-used scratch values, duplicate columns, unit columns).
    for k, sym in sorted(col_of.items()):
        if sym == ("one",):
            ops.append(("g", "one", ("out", k)))
            continue
        if retarget.get(sym) == ("out", k):
            continue
        if load["v"] + _DVE_COPY <= load["a"] + _ACT_COPY:
            load["v"] += _DVE_COPY
            ops.append(("v", "copy", ("out", k), sym))
        else:
            load["a"] += _ACT_COPY
            ops.append(("a", "copy", ("out", k), sym))

    # Compact scratch indices to the slots still written.
    live = sorted(
        {s[1] for op in ops for s in op[2:] if isinstance(s, tuple) and s[0] == "scr"}
    )
    remap = {old: i for i, old in enumerate(live)}
    ops = [
        tuple(
            ("scr", remap[s[1]])
            if isinstance(s, tuple) and len(s) == 2 and s[0] == "scr"
            else s
            for s in op
        )
        for op in ops
    ]
    return ops, len(live)


def _build(R, exps, f_max=None, inp_bufs=4, outp_bufs=4, taper=1):
    """Build the per-core Bacc graph for R rows (R divisible by P)."""
    import concourse.bass as bass  # noqa: F401
    import concourse.tile as tile
    from concourse import bacc, mybir

    L = len(exps)
    steps, col_of = _plan(exps)
    ops, nscr = _assign(steps, col_of)
    f32 = mybir.dt.float32

    nc = bacc.Bacc()
    vec = nc.declare_dram_parameter("vector_list", [R, 3], f32, isOutput=False)
    out = nc.declare_dram_parameter("out", [R, L], f32, isOutput=True)

    rows_p = R // P
    if f_max is None:
        f_max = F_MAX
    n_chunks = max(1, math.ceil(rows_p / f_max))
    base, rem = divmod(rows_p, n_chunks)
    sizes = [base + 1] * rem + [base] * (n_chunks - rem)
    for _ in range(taper):
        # Taper: split the first and last chunks so the store pipeline
        # starts earlier and drains faster.
        if len(sizes) >= 2:
            f, l = sizes[0], sizes[-1]
            sizes = [f // 2, f - f // 2] + sizes[1:-1] + [l // 2, l - l // 2]
            sizes = [s for s in sizes if s > 0]

    with tile.TileContext(nc) as tc:
        with (
            tc.tile_pool(name="inp", bufs=inp_bufs) as inp,
            tc.tile_pool(name="outp", bufs=outp_bufs) as outp,
            tc.tile_pool(name="scrp", bufs=2) as scrp,
        ):
            r0 = 0
            for F in sizes:
                rows = P * F
                tin = inp.tile([P, F, 3], f32)
                nc.scalar.dma_start(
                    tin[:],
                    vec[r0 : r0 + rows, :].rearrange("(p f) c -> p f c", p=P),
                )
                tout = outp.tile([P, F, L], f32)
                if nscr:
                    tscr = scrp.tile([P, nscr, F], f32, tag="tscr")
                else:
                    tscr = None

                def ap(sym):
                    kind, i = sym[0], sym[1]
                    if kind == "basis":
                        return tin[:, :, i]
                    if kind == "out":
                        return tout[:, :, i]
                    return tscr[:, i, :]

                for op in ops:
                    eng, kind, dst = op[0], op[1], op[2]
                    if kind == "one":
                        nc.gpsimd.memset(ap(dst), 1.0)
                    elif kind == "copy":
                        if eng == "a":
                            nc.scalar.copy(ap(dst), ap(op[3]))
                        else:
                            nc.vector.tensor_copy(ap(dst), ap(op[3]))
                    elif kind == "square":
                        nc.scalar.square(ap(dst), ap(op[3]))
                    else:
                        nc.vector.tensor_mul(ap(dst), ap(op[3]), ap(op[4]))

                nc.sync.dma_start(
                    out[r0 : r0 + rows, :].rearrange("(p f) k -> p f k", p=P),
                    tout[:],
                )
                r0 += rows
    nc.finalize()
    return nc


_CACHE = {}
_LAST_RESULT = None  # BassKernelResults of the most recent run (for profiling)


def kernel(vector_list: np.ndarray, l_list: np.ndarray) -> np.ndarray:
    from concourse.bass_utils import run_bass_kernel_spmd

    vector_list = np.ascontiguousarray(vector_list, dtype=np.float32)
    l_list = np.asarray(l_list)
    N = vector_list.shape[0]
    L = l_list.shape[0]
    exps = tuple(tuple(int(v) for v in row) for row in l_list)

    rows_unit = N_CORES * P
    n_dev = (N // rows_unit) * rows_unit
    R = n_dev // N_CORES

    outv = np.empty((N, L), dtype=np.float32)
    if R > 0:
        key = (R, exps)
        if key not in _CACHE:
            _CACHE[key] = _build(R, exps)
        nc = _CACHE[key]
        in_maps = [
            {"vector_list": vector_list[i * R : (i + 1) * R]} for i in range(N_CORES)
        ]
        res = run_bass_kernel_spmd(nc, in_maps, core_ids=list(range(N_CORES)))
        global _LAST_RESULT
        _LAST_RESULT = res
        for i in range(N_CORES):
            outv[i * R : (i + 1) * R] = res.results[i]["out"]
    if n_dev < N:
        tail = vector_list[n_dev:]
        le = np.asarray(l_list, dtype=np.float32)
        outv[n_dev:] = np.prod(
            tail[:, None, :] ** le[None, :, :], axis=-1, dtype=np.float32
        )
    return outv
